# revision 43
# baseline (speedup 1.0000x reference)
"""Distributed attention kernel for Trainium2 (8 NeuronCores), v2.

Problem: B=4, N=2048, DIM=1024, HEADS=16, DIM_HEAD=64 attention with a
[1,16,2048,2048] relative-position bias, including the four linear
projections (Wq/Wk/Wv/Wo).

Sharding (head-parallel / tensor-parallel):
  - Core c owns heads {2c, 2c+1} for ALL 8192 (b, n) tokens. Each core
    projects qh/kh/vh for its own 2 heads from the FULL x (streamed from
    DRAM), so there are NO forward collectives and the softmax exp
    stream on ScalarE starts ~15us in (vs ~110us for the Ulysses
    variant: proj -> AllToAll -> S).
  - Attention per (ic, b) combo: S^T computed transposed with
    exp(S)*exp(bias^T) (exp(bias^T) precomputed on host, bf16).
    AV runs in out-[i, d] form (lhsT = exp-tile [j, i-slice]) so all
    128 PE output partitions are used: 8320 cy/combo vs 16384 for the
    [d, i] form. A ones-column per head in vh gives the softmax
    denominator per (i, head) as a psum column.
  - Normalization happens BEFORE the backward AllToAll (DVE reciprocal
    + gpsimd tensor_scalar), the normalized [i, e] tiles are PE-
    transposed to [e, i] (fp32, via identity matmul) so the receiving
    core gets feature-major activations, and a single AllToAll (split
    in two halves: ics {0,2} fire one combo-window early) switches
    heads -> tokens for the Wo projection.
  - Combo order pairs ics {0,2} then {1,3} so bias tiles for only two
    ics are SBUF-resident at a time, and x/proj DMA demand is spread
    across the first 8 combo windows.

All matmuls bf16 (except fp32 transposes); PSUM accumulation fp32.
PSUM budget (8 banks): S 2x[128,1024] (4) + proj/transpose 2x[128,512]
(2) + AV 2x[128,512] (2).
"""
import sys

sys.path.insert(0, "/opt/trn_rl_repo")

import numpy as np

import concourse.bass as bass
import concourse.bacc as bacc
import concourse.mybir as mybir
import concourse.tile as tile
from concourse import bass_utils

NCORES = 8
B, N, DIM = 4, 2048, 1024
HEADS, DH = 16, 64
INNER = HEADS * DH  # 1024
TOKG = B * N        # 8192 global tokens
TOK = TOKG // NCORES  # 1024 output tokens per core
HPC = HEADS // NCORES  # 2 heads per core
SCALE = DH ** -0.5

BF16 = mybir.dt.bfloat16
F32 = mybir.dt.float32
NP_BF16 = mybir.dt.np(BF16)

_EXP = mybir.ActivationFunctionType.Exp


def build_nc(mock_collectives=False, repeat=1, stages="PAO", dbg_dump=False):
    nc = bacc.Bacc("TRN2", target_bir_lowering=False, debug=False,
                   num_devices=NCORES)

    xq = nc.dram_tensor("xq", [DIM, TOKG], BF16, kind="ExternalInput").ap()
    xk = nc.dram_tensor("xk", [DIM, TOKG], BF16, kind="ExternalInput").ap()
    xv = nc.dram_tensor("xv", [DIM, TOKG], BF16, kind="ExternalInput").ap()
    wq = nc.dram_tensor("wq", [DIM, HPC * DH], BF16, kind="ExternalInput").ap()
    wk = nc.dram_tensor("wk", [DIM, HPC * DH], BF16, kind="ExternalInput").ap()
    wv = nc.dram_tensor("wv", [DIM, HPC * DH], BF16, kind="ExternalInput").ap()
    wo = nc.dram_tensor("wo", [INNER, DIM], BF16, kind="ExternalInput").ap()
    # packed: expb[jb*128+j, ic*1024 + h*512 + iw] = exp(bias[h, i, j])
    expb = nc.dram_tensor("expb", [N, 2 * N], BF16, kind="ExternalInput").ap()
    out = nc.dram_tensor("out", [TOK, DIM], F32, kind="ExternalOutput").ap()
    dbg = (nc.dram_tensor("dbg", [NCORES * 128, 512], BF16,
                          kind="ExternalOutput").ap() if dbg_dump else None)

    rg = [list(range(NCORES))]

    def a2a(in_t, out_t):
        if mock_collectives:
            nc.sync.dma_start(out_t[:], in_t[:])
        else:
            nc.gpsimd.collective_compute(
                "AllToAll", mybir.AluOpType.bypass, replica_groups=rg,
                ins=[in_t.opt()], outs=[out_t.opt()])

    ident_np = np.eye(128, dtype=np.float32)
    _ident_dram = [nc.inline_tensor(ident_np, name="ident_const")]

    with tile.TileContext(nc) as tc:
        with tc.tile_pool(name="dram", bufs=1, space="DRAM") as dram, \
             tc.tile_pool(name="wpool", bufs=1) as wpool, \
             tc.tile_pool(name="xpool", bufs=10) as xpool, \
             tc.tile_pool(name="qk", bufs=1) as qk_pool, \
             tc.tile_pool(name="vh", bufs=64) as vh_pool, \
             tc.tile_pool(name="bias", bufs=8) as bias_pool, \
             tc.tile_pool(name="pexp", bufs=17) as pexp_pool, \
             tc.tile_pool(name="nrm", bufs=5) as nrm_pool, \
             tc.tile_pool(name="aot", bufs=2) as aot_pool, \
             tc.tile_pool(name="od", bufs=2) as od_pool, \
             tc.tile_pool(name="psS", bufs=2, space="PSUM") as psS, \
             tc.tile_pool(name="psP", bufs=2, space="PSUM") as psP, \
             tc.tile_pool(name="psA", bufs=2, space="PSUM") as psA:

            # -------- collective bounce buffers (internal DRAM) --------
            ao_in0 = dram.tile([NCORES * 128, 512], BF16)
            ao_in1 = dram.tile([NCORES * 128, 512], BF16)
            ao_out0 = dram.tile([NCORES * 128, 512], BF16)
            ao_out1 = dram.tile([NCORES * 128, 512], BF16)

            for _rep in range(repeat):
                # ---- weights (SWDGE, off the HWDGE path) ----
                def load_w_small(w_dram, tg):
                    wt = wpool.tile([128, 1024], BF16, tag=tg, name=tg)
                    src = w_dram[:].rearrange("(c p) f -> p c f", p=128)
                    dst = wt[:].rearrange("p (c f) -> p c f", c=8)
                    nc.gpsimd.dma_start(dst, src)
                    return wt

                wk_sb = load_w_small(wk, "wk_sb")
                wq_sb = load_w_small(wq, "wq_sb")
                wv_sb = load_w_small(wv, "wv_sb")
                ident_sb = wpool.tile([128, 128], F32, tag="ident")
                nc.scalar.dma_start(ident_sb[:], _ident_dram[0].ap()[:])

                kh_sb = {(b, qt): qk_pool.tile([128, N // 4], BF16,
                                               tag=f"kh{b}{qt}",
                                               name=f"kh{b}{qt}")
                         for b in range(B) for qt in range(4)}
                qh_sb = {(b, qt): qk_pool.tile([128, N // 4], BF16,
                                               tag=f"qh{b}{qt}",
                                               name=f"qh{b}{qt}")
                         for b in range(B) for qt in range(4)}
                # vh token-major with ones column per head:
                # [h0 d64 | 1 | h1 d64 | 1]
                vh_aug = [vh_pool.tile([128, HPC * (DH + 1)], BF16,
                                       tag="vh_aug", name=f"vh{g}")
                          for g in range(TOKG // 128)]

                def load_x(x_dram, b, half):
                    # alternate HWDGE (sync) / SWDGE (gpsimd) queues: the
                    # single HWDGE descriptor-gen unit (~625ns/DMA) would
                    # otherwise serialize the startup x stream
                    tiles = []
                    for cb in range(8):
                        xt = xpool.tile([128, 1024], BF16, tag="x_sb")
                        nc.sync.dma_start(
                            xt[:], x_dram[cb * 128:(cb + 1) * 128,
                                          b * N + half * 1024:
                                          b * N + (half + 1) * 1024])
                        tiles.append(xt)
                    return tiles

                # feature-major kh/qh proj for batch b into dst_sb[b]
                def proj_fmajor(x_dram, w_sb, dst_sb, b, halves=(0, 1)):
                    for half in halves:
                        xt = load_x(x_dram, b, half)
                        for q in range(2):
                            ps = psP.tile([128, 512], F32, tag="pj")
                            for cb in range(8):
                                nc.tensor.matmul(
                                    ps[:],
                                    w_sb[:, cb * 128:(cb + 1) * 128],
                                    xt[cb][:, q * 512:(q + 1) * 512],
                                    start=(cb == 0), stop=(cb == 7))
                            nc.vector.tensor_copy(
                                dst_sb[(b, half * 2 + q)][:], ps[:])

                # token-major vh proj for batch b, drains into vh_aug
                def proj_v(b):
                    for half in range(2):
                        xt = load_x(xv, b, half)
                        for s in range(2):  # psum slot: 4 token-chunks
                            ps = psP.tile([128, 512], F32, tag="pj")
                            for k in range(4):
                                t = s * 4 + k
                                for cb in range(8):
                                    nc.tensor.matmul(
                                        ps[:, k * 128:(k + 1) * 128],
                                        xt[cb][:, t * 128:(t + 1) * 128],
                                        wv_sb[:, cb * 128:(cb + 1) * 128],
                                        start=(cb == 0), stop=(cb == 7))
                            for k in range(4):
                                g = b * 16 + half * 8 + s * 4 + k
                                src = (ps[:, k * 128:(k + 1) * 128]
                                       .rearrange("p (h f) -> p h f", h=2))
                                dst = (vh_aug[g][:]
                                       .rearrange("p (h f) -> p h f", f=DH + 1)
                                       [:, :, 0:DH])
                                nc.vector.tensor_copy(dst, src)
                                ones = (vh_aug[g][:]
                                        .rearrange("p (h f) -> p h f",
                                                   f=DH + 1)[:, :, DH:DH + 1])
                                nc.vector.memset(ones, 1.0)

                def emit_bias(ic, quarters=range(4), tiles=None):
                    # 4 jb row-blocks concatenated along free per tile:
                    # one SWDGE DMA covers 4 jbs (512 descriptor rows)
                    if tiles is None:
                        tiles = [None] * 4
                    for jq in quarters:
                        ebt = bias_pool.tile([128, 4096], BF16, tag="eb")
                        src = (expb[jq * 512:(jq + 1) * 512,
                                    ic * 1024:(ic + 1) * 1024]
                               .rearrange("(s p) c -> p s c", p=128))
                        dst = ebt[:].rearrange("p (s c) -> p s c", s=4)
                        nc.gpsimd.dma_start(dst, src)
                        tiles[jq] = ebt
                    return tiles

                # quarter-granular proj (startup path): 8 [128,512] x tiles
                def proj_q(x_dram, w_sb, dst_tile, b, qt):
                    xt = []
                    for cb in range(8):
                        t = xpool.tile([128, 512], BF16, tag="x_q", bufs=8)
                        nc.sync.dma_start(
                            t[:], x_dram[cb * 128:(cb + 1) * 128,
                                         b * N + qt * 512:
                                         b * N + (qt + 1) * 512])
                        xt.append(t)
                    ps = psP.tile([128, 512], F32, tag="pj")
                    for cb in range(8):
                        nc.tensor.matmul(ps[:],
                                         w_sb[:, cb * 128:(cb + 1) * 128],
                                         xt[cb][:], start=(cb == 0),
                                         stop=(cb == 7))
                    nc.vector.tensor_copy(dst_tile[:], ps[:])

                def emit_S(ic, b, jb, eb_tiles):
                    ps = psS.tile([128, 1024], F32, tag="s")
                    kt = kh_sb[(b, jb // 4)]
                    qt = qh_sb[(b, ic)]
                    for h in range(HPC):
                        nc.tensor.matmul(
                            ps[:, h * 512:(h + 1) * 512],
                            kt[h * DH:(h + 1) * DH,
                               (jb % 4) * 128:(jb % 4 + 1) * 128],
                            qt[h * DH:(h + 1) * DH, :],
                            start=True, stop=True)
                    es = pexp_pool.tile([128, 1024], BF16, tag="es")
                    nc.scalar.activation(es[:], ps[:], _EXP)
                    nc.vector.tensor_mul(
                        es[:], es[:],
                        eb_tiles[jb // 4][:, (jb % 4) * 1024:
                                          (jb % 4 + 1) * 1024])
                    return es

                # AV in out-[i, d] form: av_ps[s] packs i-slices {2s, 2s+1}:
                # col layout [isl_a h0 65 | isl_a h1 65 | isl_b h0 | isl_b h1]
                def alloc_AV():
                    return [psA.tile([128, 512], F32, tag="av",
                                     name=f"av{s}") for s in range(2)]

                # one accumulation group = one (i-slice, head) region; the
                # 16 j-chunk matmuls of a group run CONSECUTIVELY: groups
                # interleaved within a PSUM bank corrupt each other on HW
                def emit_AV_group(st, gi):
                    ic, b, p_tiles, av_ps = st
                    s, k, h = gi // 4, (gi // 2) % 2, gi % 2
                    isl = 2 * s + k
                    for jb in range(16):
                        nc.tensor.matmul(
                            av_ps[s][:, k * 130 + h * 65:
                                     k * 130 + h * 65 + 65],
                            p_tiles[jb][:, h * 512 + isl * 128:
                                        h * 512 + isl * 128 + 128],
                            vh_aug[b * 16 + jb][:, h * 65:h * 65 + 65],
                            start=(jb == 0), stop=(jb == 15))

                def finish_AV(st):
                    ic, b, p_tiles, av_ps = st
                    nrm = [nrm_pool.tile([128, 128], F32, tag="nrm",
                                         name=f"nrm{i}") for i in range(4)]
                    for s in range(2):
                        rcp = nrm_pool.tile([128, 4], F32, tag="rcp")
                        den = (av_ps[s][:, 0:260]
                               .rearrange("p (k o) -> p k o", o=65)
                               [:, :, DH:DH + 1])
                        nc.vector.reciprocal(
                            rcp[:].rearrange("p (k o) -> p k o", o=1), den)
                        for k in range(2):
                            for h in range(HPC):
                                nc.vector.tensor_scalar_mul(
                                    nrm[2 * s + k][:, h * DH:(h + 1) * DH],
                                    av_ps[s][:, k * 130 + h * 65:
                                             k * 130 + h * 65 + DH],
                                    rcp[:, 2 * k + h:2 * k + h + 1])
                    psT = psP.tile([128, 512], F32, tag="pj")
                    for isl in range(4):
                        nc.tensor.transpose(
                            psT[:, isl * 128:(isl + 1) * 128],
                            nrm[isl][:], ident_sb[:])
                    aot = aot_pool.tile([128, 512], BF16, tag="aot")
                    nc.vector.tensor_copy(aot[:], psT[:])
                    t = 2 * b + ic // 2
                    dst = ao_in0 if ic % 2 == 0 else ao_in1
                    nc.sync.dma_start(dst[t * 128:(t + 1) * 128, :], aot[:])

                # ---------------- stage O ----------------
                def load_wo():
                    # xpool slots: x tiles are dead by the time wo loads
                    tiles = []
                    for eb in range(8):
                        wt = xpool.tile([128, DIM], BF16, tag="x_sb",
                                        name=f"wo{eb}")
                        nc.gpsimd.dma_start(
                            wt[:], wo[eb * 128:(eb + 1) * 128, :])
                        tiles.append(wt)
                    return tiles

                def load_aon(ao_out_x, tg):
                    # single DMA: [8*128 e, 512 i] -> [128, (8 eb, 512)]
                    at = xpool.tile([128, 8 * 512], BF16, tag=tg, name=tg,
                                    bufs=1)
                    src = ao_out_x[:].rearrange("(e p) i -> p e i", p=128)
                    dst = at[:].rearrange("p (e i) -> p e i", e=8)
                    nc.gpsimd.dma_start(dst, src)
                    return at

                def stage_O_group(X, aon, wo_sb, ts, oc):
                    ps = psP.tile([128, 512], F32, tag="pj")
                    for eb in range(8):
                        nc.tensor.matmul(
                            ps[:],
                            aon[:, eb * 512 + ts * 128:
                                eb * 512 + (ts + 1) * 128],
                            wo_sb[eb][:, oc * 512:(oc + 1) * 512],
                            start=(eb == 0), stop=(eb == 7))
                    ob = od_pool.tile([128, 512], F32, tag="od")
                    nc.vector.tensor_copy(ob[:], ps[:])
                    nc.sync.dma_start(
                        out[X * 512 + ts * 128:X * 512 + (ts + 1) * 128,
                            oc * 512:(oc + 1) * 512], ob[:])

                # ---------------- schedule ----------------
                # combo order: ic pairs {0,2} then {1,3}, b-interleaved:
                # bias for two ics SBUF-resident at a time, ao_in0 (ics
                # {0,2}) complete after combo 8 for the early half-AllToAll.
                seq = [(ic, b) for pair in ((0, 2), (1, 3))
                       for b in range(B) for ic in pair]

                # minimal work before the first exp: kh(0) + first qh chunk
                # prologue: S(0,0) jb 0-7 only needs qh(b0,q0)+kh(b0,h0); it
                # must sit BEFORE the kh-h1 proj in PE queue order or it
                # stalls on the serialized x-DMA stream. Only bias quarter 0
                # is loaded ahead of the stream.
                eb0 = emit_bias(0, quarters=(0,))
                proj_q(xq, wq_sb, qh_sb[(0, 0)], 0, 0)
                proj_q(xk, wk_sb, kh_sb[(0, 0)], 0, 0)
                proj_q(xk, wk_sb, kh_sb[(0, 1)], 0, 1)
                p0 = [emit_S(0, 0, jb, eb0) for jb in range(4)]
                eb_by_ic = {0: emit_bias(0, quarters=(1, 2, 3), tiles=eb0)}
                p0 += [emit_S(0, 0, jb, eb0) for jb in range(4, 8)]
                proj_fmajor(xk, wk_sb, kh_sb, 0, halves=(1,))
                p0 += [emit_S(0, 0, jb, eb0) for jb in range(8, 16)]
                prev = (0, 0, p0, None)

                proj_fmajor(xq, wq_sb, qh_sb, 0, halves=(1,))
                proj_q(xq, wq_sb, qh_sb[(0, 1)], 0, 1)
                proj_v(0)

                # interleaved proj work: chunk i emitted by end of window
                # deadline[i]; batch b first used at window 2b (S) / 2b+1
                # (AV).
                proj_chunks = [
                    (1, lambda: proj_fmajor(xk, wk_sb, kh_sb, 1)),
                    (1, lambda: proj_fmajor(xq, wq_sb, qh_sb, 1)),
                    (2, lambda: proj_v(1)),
                    (3, lambda: proj_fmajor(xk, wk_sb, kh_sb, 2)),
                    (3, lambda: proj_fmajor(xq, wq_sb, qh_sb, 2)),
                    (4, lambda: proj_v(2)),
                    (5, lambda: proj_fmajor(xk, wk_sb, kh_sb, 3)),
                    (5, lambda: proj_fmajor(xq, wq_sb, qh_sb, 3)),
                    (6, lambda: proj_v(3)),
                ]

                wo_sb = None
                aon0 = None
                o0_groups = [(ts, oc) for ts in range(4) for oc in range(2)]
                for n in range(1, len(seq) + 1):
                    prev = (prev[0], prev[1], prev[2], alloc_AV())
                    fin_ic_b = (prev[0], prev[1])
                    if n < len(seq):
                        ic, b = seq[n]
                        if ic not in eb_by_ic:
                            eb_by_ic[ic] = emit_bias(ic)
                        p_cur = []
                        for jb in range(16):
                            p_cur.append(emit_S(ic, b, jb, eb_by_ic[ic]))
                            if jb % 2 == 1:
                                emit_AV_group(prev, jb // 2)
                        finish_AV(prev)
                        prev = (ic, b, p_cur, None)
                        while proj_chunks and proj_chunks[0][0] <= n:
                            proj_chunks.pop(0)[1]()
                        if False and n >= 13 and aon0 is not None and "O" in stages:
                            # ride phase-2 PE slack with X0 out-proj groups
                            for _ in range(3):
                                if o0_groups:
                                    ts, oc = o0_groups.pop(0)
                                    stage_O_group(0, aon0, wo_sb, ts, oc)
                    else:
                        for gi in range(8):
                            emit_AV_group(prev, gi)
                        finish_AV(prev)
                        a2a(ao_in1, ao_out1)
                        if "O" in stages:
                            aon1 = load_aon(ao_out1, "aon1")
                            for ts, oc in o0_groups:
                                stage_O_group(0, aon0, wo_sb, ts, oc)
                            for ts in range(4):
                                for oc in range(2):
                                    stage_O_group(1, aon1, wo_sb, ts, oc)
                    if fin_ic_b == (2, 3):
                        # all ic {0,2} blocks written -> ship first half
                        if dbg_dump:
                            nc.sync.dma_start(dbg[:], ao_in0[:])
                        a2a(ao_in0, ao_out0)
                        wo_sb = load_wo()
                        aon0 = load_aon(ao_out0, "aon0")

    nc.compile()
    return nc


def make_in_maps(q, k, v, rel_pos_bias, Wq, Wk, Wv, Wo):
    """Host-side sharding: transposes, bf16 casts, exp(bias) packing."""
    q = np.asarray(q, np.float32).reshape(TOKG, DIM)
    k = np.asarray(k, np.float32).reshape(TOKG, DIM)
    v = np.asarray(v, np.float32).reshape(TOKG, DIM)
    bias = np.asarray(rel_pos_bias, np.float32)
    xq_t = np.ascontiguousarray(q.T).astype(NP_BF16)
    xk_t = np.ascontiguousarray(k.T).astype(NP_BF16)
    xv_t = np.ascontiguousarray(v.T).astype(NP_BF16)
    wq_t = np.ascontiguousarray((np.asarray(Wq) * SCALE).T).astype(NP_BF16)
    wk_t = np.ascontiguousarray(np.asarray(Wk).T).astype(NP_BF16)
    wv_t = np.ascontiguousarray(np.asarray(Wv).T).astype(NP_BF16)
    wo_t = np.ascontiguousarray(np.asarray(Wo).T).astype(NP_BF16)
    in_maps = []
    for c in range(NCORES):
        hs = slice(HPC * DH * c, HPC * DH * (c + 1))
        # [h, i, j] -> exp -> packed [j, (ic, h, iw)] with i = ic*512 + iw
        eb = np.exp(bias[0, HPC * c:HPC * (c + 1)])          # [h, i, j]
        eb = eb.transpose(2, 1, 0)                            # [j, i, h]
        eb = eb.reshape(N, 4, 512, 2).transpose(0, 1, 3, 2)   # [j, ic, h, iw]
        in_maps.append({
            "xq": xq_t, "xk": xk_t, "xv": xv_t,
            "wq": np.ascontiguousarray(wq_t[:, hs]),
            "wk": np.ascontiguousarray(wk_t[:, hs]),
            "wv": np.ascontiguousarray(wv_t[:, hs]),
            "wo": wo_t,
            "expb": np.ascontiguousarray(eb.reshape(N, 2 * N)).astype(NP_BF16),
        })
    return in_maps


_NC_CACHE = None


def kernel(q, k, v, rel_pos_bias, Wq, Wk, Wv, Wo):
    global _NC_CACHE
    if _NC_CACHE is None:
        _NC_CACHE = build_nc()
    nc = _NC_CACHE
    in_maps = make_in_maps(q, k, v, rel_pos_bias, Wq, Wk, Wv, Wo)
    res = bass_utils.run_bass_kernel_spmd(nc, in_maps,
                                          core_ids=list(range(NCORES)))
    out = np.empty((B, N, DIM), np.float32)
    for c in range(NCORES):
        b, half = c // 2, c % 2
        out[b, half * TOK:(half + 1) * TOK, :] = res.results[c]["out"]
    return out


# revision 44
# speedup vs baseline: 1.7159x; 1.7159x over previous
"""Distributed attention kernel for Trainium2 (8 NeuronCores), v2.

Problem: B=4, N=2048, DIM=1024, HEADS=16, DIM_HEAD=64 attention with a
[1,16,2048,2048] relative-position bias, including the four linear
projections (Wq/Wk/Wv/Wo).

Sharding (head-parallel / tensor-parallel):
  - Core c owns heads {2c, 2c+1} for ALL 8192 (b, n) tokens. Each core
    projects qh/kh/vh for its own 2 heads from the FULL x (streamed from
    DRAM), so there are NO forward collectives and the softmax exp
    stream on ScalarE starts ~15us in (vs ~110us for the Ulysses
    variant: proj -> AllToAll -> S).
  - Attention per (ic, b) combo: S^T computed transposed with
    exp(S)*exp(bias^T) (exp(bias^T) precomputed on host, bf16).
    AV runs in out-[i, d] form (lhsT = exp-tile [j, i-slice]) so all
    128 PE output partitions are used: 8320 cy/combo vs 16384 for the
    [d, i] form. A ones-column per head in vh gives the softmax
    denominator per (i, head) as a psum column.
  - Normalization happens BEFORE the backward AllToAll (DVE reciprocal
    + gpsimd tensor_scalar), the normalized [i, e] tiles are PE-
    transposed to [e, i] (fp32, via identity matmul) so the receiving
    core gets feature-major activations, and a single AllToAll (split
    in two halves: ics {0,2} fire one combo-window early) switches
    heads -> tokens for the Wo projection.
  - Combo order pairs ics {0,2} then {1,3} so bias tiles for only two
    ics are SBUF-resident at a time, and x/proj DMA demand is spread
    across the first 8 combo windows.

All matmuls bf16 (except fp32 transposes); PSUM accumulation fp32.
PSUM budget (8 banks): S 2x[128,1024] (4) + proj/transpose 2x[128,512]
(2) + AV 2x[128,512] (2).
"""
import sys

sys.path.insert(0, "/opt/trn_rl_repo")

import numpy as np

import concourse.bass as bass
import concourse.bacc as bacc
import concourse.mybir as mybir
import concourse.tile as tile
from concourse import bass_utils

NCORES = 8
B, N, DIM = 4, 2048, 1024
HEADS, DH = 16, 64
INNER = HEADS * DH  # 1024
TOKG = B * N        # 8192 global tokens
TOK = TOKG // NCORES  # 1024 output tokens per core
HPC = HEADS // NCORES  # 2 heads per core
SCALE = DH ** -0.5

BF16 = mybir.dt.bfloat16
F32 = mybir.dt.float32
NP_BF16 = mybir.dt.np(BF16)

_EXP = mybir.ActivationFunctionType.Exp


def build_nc(mock_collectives=False, repeat=1, stages="PAO", dbg_dump=False):
    nc = bacc.Bacc("TRN2", target_bir_lowering=False, debug=False,
                   num_devices=NCORES)

    xq = nc.dram_tensor("xq", [DIM, TOKG], BF16, kind="ExternalInput").ap()
    xk = nc.dram_tensor("xk", [DIM, TOKG], BF16, kind="ExternalInput").ap()
    xv = nc.dram_tensor("xv", [DIM, TOKG], BF16, kind="ExternalInput").ap()
    wq = nc.dram_tensor("wq", [DIM, HPC * DH], BF16, kind="ExternalInput").ap()
    wk = nc.dram_tensor("wk", [DIM, HPC * DH], BF16, kind="ExternalInput").ap()
    wv = nc.dram_tensor("wv", [DIM, HPC * DH], BF16, kind="ExternalInput").ap()
    wo = nc.dram_tensor("wo", [INNER, DIM], BF16, kind="ExternalInput").ap()
    # packed: expb[jb*128+j, ic*1024 + h*512 + iw] = exp(bias[h, i, j])
    expb = nc.dram_tensor("expb", [N, 2 * N], BF16, kind="ExternalInput").ap()
    out = nc.dram_tensor("out", [TOK, DIM], F32, kind="ExternalOutput").ap()
    dbg = (nc.dram_tensor("dbg", [NCORES * 128, 512], BF16,
                          kind="ExternalOutput").ap() if dbg_dump else None)

    rg = [list(range(NCORES))]

    def a2a(in_t, out_t):
        if mock_collectives:
            nc.sync.dma_start(out_t[:], in_t[:])
        else:
            nc.gpsimd.collective_compute(
                "AllToAll", mybir.AluOpType.bypass, replica_groups=rg,
                ins=[in_t.opt()], outs=[out_t.opt()])

    ident_np = np.eye(128, dtype=np.float32)
    _ident_dram = [nc.inline_tensor(ident_np, name="ident_const")]

    with tile.TileContext(nc) as tc:
        with tc.tile_pool(name="dram", bufs=1, space="DRAM") as dram, \
             tc.tile_pool(name="wpool", bufs=1) as wpool, \
             tc.tile_pool(name="xpool", bufs=10) as xpool, \
             tc.tile_pool(name="qk", bufs=1) as qk_pool, \
             tc.tile_pool(name="vh", bufs=64) as vh_pool, \
             tc.tile_pool(name="bias", bufs=8) as bias_pool, \
             tc.tile_pool(name="pexp", bufs=17) as pexp_pool, \
             tc.tile_pool(name="nrm", bufs=5) as nrm_pool, \
             tc.tile_pool(name="aot", bufs=2) as aot_pool, \
             tc.tile_pool(name="od", bufs=2) as od_pool, \
             tc.tile_pool(name="psS", bufs=2, space="PSUM") as psS, \
             tc.tile_pool(name="psP", bufs=2, space="PSUM") as psP, \
             tc.tile_pool(name="psA", bufs=2, space="PSUM") as psA:

            # -------- collective bounce buffers (internal DRAM) --------
            ao_in0 = dram.tile([NCORES * 128, 512], BF16)
            ao_in1 = dram.tile([NCORES * 128, 512], BF16)
            ao_out0 = dram.tile([NCORES * 128, 512], BF16)
            ao_out1 = dram.tile([NCORES * 128, 512], BF16)

            for _rep in range(repeat):
                # ---- weights (SWDGE, off the HWDGE path) ----
                def load_w_small(w_dram, tg):
                    wt = wpool.tile([128, 1024], BF16, tag=tg, name=tg)
                    src = w_dram[:].rearrange("(c p) f -> p c f", p=128)
                    dst = wt[:].rearrange("p (c f) -> p c f", c=8)
                    nc.gpsimd.dma_start(dst, src)
                    return wt

                wk_sb = load_w_small(wk, "wk_sb")
                wq_sb = load_w_small(wq, "wq_sb")
                wv_sb = load_w_small(wv, "wv_sb")
                ident_sb = wpool.tile([128, 128], F32, tag="ident")
                nc.scalar.dma_start(ident_sb[:], _ident_dram[0].ap()[:])

                kh_sb = {(b, qt): qk_pool.tile([128, N // 4], BF16,
                                               tag=f"kh{b}{qt}",
                                               name=f"kh{b}{qt}")
                         for b in range(B) for qt in range(4)}
                qh_sb = {(b, qt): qk_pool.tile([128, N // 4], BF16,
                                               tag=f"qh{b}{qt}",
                                               name=f"qh{b}{qt}")
                         for b in range(B) for qt in range(4)}
                # vh token-major with ones column per head:
                # [h0 d64 | 1 | h1 d64 | 1]
                vh_aug = [vh_pool.tile([128, HPC * (DH + 1)], BF16,
                                       tag="vh_aug", name=f"vh{g}")
                          for g in range(TOKG // 128)]

                def load_x(x_dram, b, half):
                    # alternate HWDGE (sync) / SWDGE (gpsimd) queues: the
                    # single HWDGE descriptor-gen unit (~625ns/DMA) would
                    # otherwise serialize the startup x stream
                    tiles = []
                    for cb in range(8):
                        xt = xpool.tile([128, 1024], BF16, tag="x_sb")
                        nc.sync.dma_start(
                            xt[:], x_dram[cb * 128:(cb + 1) * 128,
                                          b * N + half * 1024:
                                          b * N + (half + 1) * 1024])
                        tiles.append(xt)
                    return tiles

                # feature-major kh/qh proj for batch b into dst_sb[b]
                def proj_fmajor(x_dram, w_sb, dst_sb, b, halves=(0, 1)):
                    for half in halves:
                        xt = load_x(x_dram, b, half)
                        for q in range(2):
                            ps = psP.tile([128, 512], F32, tag="pj")
                            for cb in range(8):
                                nc.tensor.matmul(
                                    ps[:],
                                    w_sb[:, cb * 128:(cb + 1) * 128],
                                    xt[cb][:, q * 512:(q + 1) * 512],
                                    start=(cb == 0), stop=(cb == 7))
                            nc.vector.tensor_copy(
                                dst_sb[(b, half * 2 + q)][:], ps[:])

                # token-major vh proj for batch b, drains into vh_aug
                def proj_v(b):
                    for half in range(2):
                        xt = load_x(xv, b, half)
                        for s in range(2):  # psum slot: 4 token-chunks
                            ps = psP.tile([128, 512], F32, tag="pj")
                            for k in range(4):
                                t = s * 4 + k
                                for cb in range(8):
                                    nc.tensor.matmul(
                                        ps[:, k * 128:(k + 1) * 128],
                                        xt[cb][:, t * 128:(t + 1) * 128],
                                        wv_sb[:, cb * 128:(cb + 1) * 128],
                                        start=(cb == 0), stop=(cb == 7))
                            for k in range(4):
                                g = b * 16 + half * 8 + s * 4 + k
                                src = (ps[:, k * 128:(k + 1) * 128]
                                       .rearrange("p (h f) -> p h f", h=2))
                                dst = (vh_aug[g][:]
                                       .rearrange("p (h f) -> p h f", f=DH + 1)
                                       [:, :, 0:DH])
                                nc.vector.tensor_copy(dst, src)
                                ones = (vh_aug[g][:]
                                        .rearrange("p (h f) -> p h f",
                                                   f=DH + 1)[:, :, DH:DH + 1])
                                nc.vector.memset(ones, 1.0)

                def emit_bias(ic, quarters=range(4), tiles=None):
                    # 4 jb row-blocks concatenated along free per tile:
                    # one SWDGE DMA covers 4 jbs (512 descriptor rows)
                    if tiles is None:
                        tiles = [None] * 4
                    for jq in quarters:
                        ebt = bias_pool.tile([128, 4096], BF16, tag="eb")
                        src = (expb[jq * 512:(jq + 1) * 512,
                                    ic * 1024:(ic + 1) * 1024]
                               .rearrange("(s p) c -> p s c", p=128))
                        dst = ebt[:].rearrange("p (s c) -> p s c", s=4)
                        nc.gpsimd.dma_start(dst, src)
                        tiles[jq] = ebt
                    return tiles

                # quarter-granular proj (startup path): 8 [128,512] x tiles
                def proj_q(x_dram, w_sb, dst_tile, b, qt):
                    xt = []
                    for cb in range(8):
                        t = xpool.tile([128, 512], BF16, tag="x_q", bufs=8)
                        nc.sync.dma_start(
                            t[:], x_dram[cb * 128:(cb + 1) * 128,
                                         b * N + qt * 512:
                                         b * N + (qt + 1) * 512])
                        xt.append(t)
                    ps = psP.tile([128, 512], F32, tag="pj")
                    for cb in range(8):
                        nc.tensor.matmul(ps[:],
                                         w_sb[:, cb * 128:(cb + 1) * 128],
                                         xt[cb][:], start=(cb == 0),
                                         stop=(cb == 7))
                    nc.vector.tensor_copy(dst_tile[:], ps[:])

                def emit_S(ic, b, jb, eb_tiles):
                    ps = psS.tile([128, 1024], F32, tag="s")
                    kt = kh_sb[(b, jb // 4)]
                    qt = qh_sb[(b, ic)]
                    for h in range(HPC):
                        nc.tensor.matmul(
                            ps[:, h * 512:(h + 1) * 512],
                            kt[h * DH:(h + 1) * DH,
                               (jb % 4) * 128:(jb % 4 + 1) * 128],
                            qt[h * DH:(h + 1) * DH, :],
                            start=True, stop=True)
                    es = pexp_pool.tile([128, 1024], BF16, tag="es")
                    nc.scalar.activation(es[:], ps[:], _EXP)
                    nc.vector.tensor_mul(
                        es[:], es[:],
                        eb_tiles[jb // 4][:, (jb % 4) * 1024:
                                          (jb % 4 + 1) * 1024])
                    return es

                # AV in out-[i, d] form: av_ps[s] packs i-slices {2s, 2s+1}:
                # col layout [isl_a h0 65 | isl_a h1 65 | isl_b h0 | isl_b h1]
                def alloc_AV():
                    return [psA.tile([128, 512], F32, tag="av",
                                     name=f"av{s}") for s in range(2)]

                # one accumulation group = one (i-slice, head) region; the
                # 16 j-chunk matmuls of a group run CONSECUTIVELY: groups
                # interleaved within a PSUM bank corrupt each other on HW
                def emit_AV_group(st, gi):
                    ic, b, p_tiles, av_ps = st
                    s, k, h = gi // 4, (gi // 2) % 2, gi % 2
                    isl = 2 * s + k
                    for jb in range(16):
                        nc.tensor.matmul(
                            av_ps[s][:, k * 130 + h * 65:
                                     k * 130 + h * 65 + 65],
                            p_tiles[jb][:, h * 512 + isl * 128:
                                        h * 512 + isl * 128 + 128],
                            vh_aug[b * 16 + jb][:, h * 65:h * 65 + 65],
                            start=(jb == 0), stop=(jb == 15))

                def finish_AV(st):
                    ic, b, p_tiles, av_ps = st
                    nrm = [nrm_pool.tile([128, 128], F32, tag="nrm",
                                         name=f"nrm{i}") for i in range(4)]
                    for s in range(2):
                        rcp = nrm_pool.tile([128, 4], F32, tag="rcp")
                        den = (av_ps[s][:, 0:260]
                               .rearrange("p (k o) -> p k o", o=65)
                               [:, :, DH:DH + 1])
                        nc.vector.reciprocal(
                            rcp[:].rearrange("p (k o) -> p k o", o=1), den)
                        for k in range(2):
                            for h in range(HPC):
                                nc.vector.tensor_scalar_mul(
                                    nrm[2 * s + k][:, h * DH:(h + 1) * DH],
                                    av_ps[s][:, k * 130 + h * 65:
                                             k * 130 + h * 65 + DH],
                                    rcp[:, 2 * k + h:2 * k + h + 1])
                    psT = psP.tile([128, 512], F32, tag="pj")
                    for isl in range(4):
                        nc.tensor.transpose(
                            psT[:, isl * 128:(isl + 1) * 128],
                            nrm[isl][:], ident_sb[:])
                    aot = aot_pool.tile([128, 512], BF16, tag="aot")
                    nc.vector.tensor_copy(aot[:], psT[:])
                    t = 2 * b + ic // 2
                    dst = ao_in0 if ic % 2 == 0 else ao_in1
                    nc.gpsimd.dma_start(dst[t * 128:(t + 1) * 128, :], aot[:])

                # ---------------- stage O ----------------
                def load_wo():
                    # xpool slots: x tiles are dead by the time wo loads
                    tiles = []
                    for eb in range(8):
                        wt = xpool.tile([128, DIM], BF16, tag="x_sb",
                                        name=f"wo{eb}")
                        nc.gpsimd.dma_start(
                            wt[:], wo[eb * 128:(eb + 1) * 128, :])
                        tiles.append(wt)
                    return tiles

                def load_aon(ao_out_x, tg):
                    # single DMA: [8*128 e, 512 i] -> [128, (8 eb, 512)]
                    at = xpool.tile([128, 8 * 512], BF16, tag=tg, name=tg,
                                    bufs=1)
                    src = ao_out_x[:].rearrange("(e p) i -> p e i", p=128)
                    dst = at[:].rearrange("p (e i) -> p e i", e=8)
                    nc.gpsimd.dma_start(dst, src)
                    return at

                def stage_O_group(X, aon, wo_sb, ts, oc):
                    ps = psP.tile([128, 512], F32, tag="pj")
                    for eb in range(8):
                        nc.tensor.matmul(
                            ps[:],
                            aon[:, eb * 512 + ts * 128:
                                eb * 512 + (ts + 1) * 128],
                            wo_sb[eb][:, oc * 512:(oc + 1) * 512],
                            start=(eb == 0), stop=(eb == 7))
                    ob = od_pool.tile([128, 512], F32, tag="od")
                    nc.vector.tensor_copy(ob[:], ps[:])
                    nc.gpsimd.dma_start(
                        out[X * 512 + ts * 128:X * 512 + (ts + 1) * 128,
                            oc * 512:(oc + 1) * 512], ob[:])

                # ---------------- schedule ----------------
                # combo order: ic pairs {0,2} then {1,3}, b-interleaved:
                # bias for two ics SBUF-resident at a time, ao_in0 (ics
                # {0,2}) complete after combo 8 for the early half-AllToAll.
                seq = [(ic, b) for pair in ((0, 2), (1, 3))
                       for b in range(B) for ic in pair]

                # minimal work before the first exp: kh(0) + first qh chunk
                # prologue: S(0,0) jb 0-7 only needs qh(b0,q0)+kh(b0,h0); it
                # must sit BEFORE the kh-h1 proj in PE queue order or it
                # stalls on the serialized x-DMA stream. Only bias quarter 0
                # is loaded ahead of the stream.
                eb0 = emit_bias(0, quarters=(0,))
                proj_q(xq, wq_sb, qh_sb[(0, 0)], 0, 0)
                proj_q(xk, wk_sb, kh_sb[(0, 0)], 0, 0)
                proj_q(xk, wk_sb, kh_sb[(0, 1)], 0, 1)
                p0 = [emit_S(0, 0, jb, eb0) for jb in range(4)]
                eb_by_ic = {0: emit_bias(0, quarters=(1, 2, 3), tiles=eb0)}
                p0 += [emit_S(0, 0, jb, eb0) for jb in range(4, 8)]
                proj_fmajor(xk, wk_sb, kh_sb, 0, halves=(1,))
                p0 += [emit_S(0, 0, jb, eb0) for jb in range(8, 16)]
                prev = (0, 0, p0, None)

                proj_fmajor(xq, wq_sb, qh_sb, 0, halves=(1,))
                proj_q(xq, wq_sb, qh_sb[(0, 1)], 0, 1)
                proj_v(0)

                # interleaved proj work: chunk i emitted by end of window
                # deadline[i]; batch b first used at window 2b (S) / 2b+1
                # (AV).
                proj_chunks = [
                    (1, lambda: proj_fmajor(xk, wk_sb, kh_sb, 1)),
                    (1, lambda: proj_fmajor(xq, wq_sb, qh_sb, 1)),
                    (2, lambda: proj_v(1)),
                    (3, lambda: proj_fmajor(xk, wk_sb, kh_sb, 2)),
                    (3, lambda: proj_fmajor(xq, wq_sb, qh_sb, 2)),
                    (4, lambda: proj_v(2)),
                    (5, lambda: proj_fmajor(xk, wk_sb, kh_sb, 3)),
                    (5, lambda: proj_fmajor(xq, wq_sb, qh_sb, 3)),
                    (6, lambda: proj_v(3)),
                ]

                wo_sb = None
                aon0 = None
                o0_groups = [(ts, oc) for ts in range(4) for oc in range(2)]
                for n in range(1, len(seq) + 1):
                    prev = (prev[0], prev[1], prev[2], alloc_AV())
                    fin_ic_b = (prev[0], prev[1])
                    if n < len(seq):
                        ic, b = seq[n]
                        if ic not in eb_by_ic:
                            eb_by_ic[ic] = emit_bias(ic)
                        p_cur = []
                        for jb in range(16):
                            p_cur.append(emit_S(ic, b, jb, eb_by_ic[ic]))
                            if jb % 2 == 1:
                                emit_AV_group(prev, jb // 2)
                        finish_AV(prev)
                        prev = (ic, b, p_cur, None)
                        while proj_chunks and proj_chunks[0][0] <= n:
                            proj_chunks.pop(0)[1]()
                        if False and n >= 13 and aon0 is not None and "O" in stages:
                            # ride phase-2 PE slack with X0 out-proj groups
                            for _ in range(3):
                                if o0_groups:
                                    ts, oc = o0_groups.pop(0)
                                    stage_O_group(0, aon0, wo_sb, ts, oc)
                    else:
                        for gi in range(8):
                            emit_AV_group(prev, gi)
                        finish_AV(prev)
                        a2a(ao_in1, ao_out1)
                        if "O" in stages:
                            aon1 = load_aon(ao_out1, "aon1")
                            for ts, oc in o0_groups:
                                stage_O_group(0, aon0, wo_sb, ts, oc)
                            for ts in range(4):
                                for oc in range(2):
                                    stage_O_group(1, aon1, wo_sb, ts, oc)
                    if fin_ic_b == (2, 3):
                        # all ic {0,2} blocks written -> ship first half
                        if dbg_dump:
                            nc.sync.dma_start(dbg[:], ao_in0[:])
                        a2a(ao_in0, ao_out0)
                        wo_sb = load_wo()
                        aon0 = load_aon(ao_out0, "aon0")

    nc.compile()
    return nc


def make_in_maps(q, k, v, rel_pos_bias, Wq, Wk, Wv, Wo):
    """Host-side sharding: transposes, bf16 casts, exp(bias) packing."""
    q = np.asarray(q, np.float32).reshape(TOKG, DIM)
    k = np.asarray(k, np.float32).reshape(TOKG, DIM)
    v = np.asarray(v, np.float32).reshape(TOKG, DIM)
    bias = np.asarray(rel_pos_bias, np.float32)
    xq_t = np.ascontiguousarray(q.T).astype(NP_BF16)
    xk_t = np.ascontiguousarray(k.T).astype(NP_BF16)
    xv_t = np.ascontiguousarray(v.T).astype(NP_BF16)
    wq_t = np.ascontiguousarray((np.asarray(Wq) * SCALE).T).astype(NP_BF16)
    wk_t = np.ascontiguousarray(np.asarray(Wk).T).astype(NP_BF16)
    wv_t = np.ascontiguousarray(np.asarray(Wv).T).astype(NP_BF16)
    wo_t = np.ascontiguousarray(np.asarray(Wo).T).astype(NP_BF16)
    in_maps = []
    for c in range(NCORES):
        hs = slice(HPC * DH * c, HPC * DH * (c + 1))
        # [h, i, j] -> exp -> packed [j, (ic, h, iw)] with i = ic*512 + iw
        eb = np.exp(bias[0, HPC * c:HPC * (c + 1)])          # [h, i, j]
        eb = eb.transpose(2, 1, 0)                            # [j, i, h]
        eb = eb.reshape(N, 4, 512, 2).transpose(0, 1, 3, 2)   # [j, ic, h, iw]
        in_maps.append({
            "xq": xq_t, "xk": xk_t, "xv": xv_t,
            "wq": np.ascontiguousarray(wq_t[:, hs]),
            "wk": np.ascontiguousarray(wk_t[:, hs]),
            "wv": np.ascontiguousarray(wv_t[:, hs]),
            "wo": wo_t,
            "expb": np.ascontiguousarray(eb.reshape(N, 2 * N)).astype(NP_BF16),
        })
    return in_maps


_NC_CACHE = None


def kernel(q, k, v, rel_pos_bias, Wq, Wk, Wv, Wo):
    global _NC_CACHE
    if _NC_CACHE is None:
        _NC_CACHE = build_nc()
    nc = _NC_CACHE
    in_maps = make_in_maps(q, k, v, rel_pos_bias, Wq, Wk, Wv, Wo)
    res = bass_utils.run_bass_kernel_spmd(nc, in_maps,
                                          core_ids=list(range(NCORES)))
    out = np.empty((B, N, DIM), np.float32)
    for c in range(NCORES):
        b, half = c // 2, c % 2
        out[b, half * TOK:(half + 1) * TOK, :] = res.results[c]["out"]
    return out


# revision 45
# speedup vs baseline: 2.0825x; 1.2136x over previous
"""Distributed attention kernel for Trainium2 (8 NeuronCores), v2.

Problem: B=4, N=2048, DIM=1024, HEADS=16, DIM_HEAD=64 attention with a
[1,16,2048,2048] relative-position bias, including the four linear
projections (Wq/Wk/Wv/Wo).

Sharding (head-parallel / tensor-parallel):
  - Core c owns heads {2c, 2c+1} for ALL 8192 (b, n) tokens. Each core
    projects qh/kh/vh for its own 2 heads from the FULL x (streamed from
    DRAM), so there are NO forward collectives and the softmax exp
    stream on ScalarE starts ~15us in (vs ~110us for the Ulysses
    variant: proj -> AllToAll -> S).
  - Attention per (ic, b) combo: S^T computed transposed with
    exp(S)*exp(bias^T) (exp(bias^T) precomputed on host, bf16).
    AV runs in out-[i, d] form (lhsT = exp-tile [j, i-slice]) so all
    128 PE output partitions are used: 8320 cy/combo vs 16384 for the
    [d, i] form. A ones-column per head in vh gives the softmax
    denominator per (i, head) as a psum column.
  - Normalization happens BEFORE the backward AllToAll (DVE reciprocal
    + gpsimd tensor_scalar), the normalized [i, e] tiles are PE-
    transposed to [e, i] (fp32, via identity matmul) so the receiving
    core gets feature-major activations, and a single AllToAll (split
    in two halves: ics {0,2} fire one combo-window early) switches
    heads -> tokens for the Wo projection.
  - Combo order pairs ics {0,2} then {1,3} so bias tiles for only two
    ics are SBUF-resident at a time, and x/proj DMA demand is spread
    across the first 8 combo windows.

All matmuls bf16 (except fp32 transposes); PSUM accumulation fp32.
PSUM budget (8 banks): S 2x[128,1024] (4) + proj/transpose 2x[128,512]
(2) + AV 2x[128,512] (2).
"""
import sys

sys.path.insert(0, "/opt/trn_rl_repo")

import numpy as np

import concourse.bass as bass
import concourse.bacc as bacc
import concourse.mybir as mybir
import concourse.tile as tile
from concourse import bass_utils

NCORES = 8
B, N, DIM = 4, 2048, 1024
HEADS, DH = 16, 64
INNER = HEADS * DH  # 1024
TOKG = B * N        # 8192 global tokens
TOK = TOKG // NCORES  # 1024 output tokens per core
HPC = HEADS // NCORES  # 2 heads per core
SCALE = DH ** -0.5

BF16 = mybir.dt.bfloat16
F32 = mybir.dt.float32
NP_BF16 = mybir.dt.np(BF16)

_EXP = mybir.ActivationFunctionType.Exp


def build_nc(mock_collectives=False, repeat=1, stages="PAO", dbg_dump=False):
    nc = bacc.Bacc("TRN2", target_bir_lowering=False, debug=False,
                   num_devices=NCORES)

    xq = nc.dram_tensor("xq", [DIM, TOKG], BF16, kind="ExternalInput").ap()
    xk = nc.dram_tensor("xk", [DIM, TOKG], BF16, kind="ExternalInput").ap()
    xv = nc.dram_tensor("xv", [DIM, TOKG], BF16, kind="ExternalInput").ap()
    wq = nc.dram_tensor("wq", [DIM, HPC * DH], BF16, kind="ExternalInput").ap()
    wk = nc.dram_tensor("wk", [DIM, HPC * DH], BF16, kind="ExternalInput").ap()
    wv = nc.dram_tensor("wv", [DIM, HPC * DH], BF16, kind="ExternalInput").ap()
    wo = nc.dram_tensor("wo", [INNER, DIM], BF16, kind="ExternalInput").ap()
    # packed: expb[jb*128+j, ic*1024 + h*512 + iw] = exp(bias[h, i, j])
    expb = nc.dram_tensor("expb", [N, 2 * N], BF16, kind="ExternalInput").ap()
    out = nc.dram_tensor("out", [TOK, DIM], F32, kind="ExternalOutput").ap()
    dbg = (nc.dram_tensor("dbg", [NCORES * 128, 512], BF16,
                          kind="ExternalOutput").ap() if dbg_dump else None)

    rg = [list(range(NCORES))]

    def a2a(in_t, out_t):
        if mock_collectives:
            nc.sync.dma_start(out_t[:], in_t[:])
        else:
            nc.gpsimd.collective_compute(
                "AllToAll", mybir.AluOpType.bypass, replica_groups=rg,
                ins=[in_t.opt()], outs=[out_t.opt()])

    ident_np = np.eye(128, dtype=np.float32)
    _ident_dram = [nc.inline_tensor(ident_np, name="ident_const")]

    with tile.TileContext(nc) as tc:
        with tc.tile_pool(name="dram", bufs=1, space="DRAM") as dram, \
             tc.tile_pool(name="wpool", bufs=1) as wpool, \
             tc.tile_pool(name="xpool", bufs=10) as xpool, \
             tc.tile_pool(name="qk", bufs=1) as qk_pool, \
             tc.tile_pool(name="vh", bufs=64) as vh_pool, \
             tc.tile_pool(name="bias", bufs=8) as bias_pool, \
             tc.tile_pool(name="pexp", bufs=17) as pexp_pool, \
             tc.tile_pool(name="nrm", bufs=5) as nrm_pool, \
             tc.tile_pool(name="aot", bufs=2) as aot_pool, \
             tc.tile_pool(name="od", bufs=2) as od_pool, \
             tc.tile_pool(name="psS", bufs=2, space="PSUM") as psS, \
             tc.tile_pool(name="psP", bufs=2, space="PSUM") as psP, \
             tc.tile_pool(name="psA", bufs=2, space="PSUM") as psA:

            # -------- collective bounce buffers (internal DRAM) --------
            ao_in0 = dram.tile([NCORES * 128, 512], BF16)
            ao_in1 = dram.tile([NCORES * 128, 512], BF16)
            ao_out0 = dram.tile([NCORES * 128, 512], BF16)
            ao_out1 = dram.tile([NCORES * 128, 512], BF16)

            for _rep in range(repeat):
                # ---- weights (SWDGE, off the HWDGE path) ----
                def load_w_small(w_dram, tg):
                    wt = wpool.tile([128, 1024], BF16, tag=tg, name=tg)
                    src = w_dram[:].rearrange("(c p) f -> p c f", p=128)
                    dst = wt[:].rearrange("p (c f) -> p c f", c=8)
                    nc.gpsimd.dma_start(dst, src)
                    return wt

                wk_sb = load_w_small(wk, "wk_sb")
                wq_sb = load_w_small(wq, "wq_sb")
                wv_sb = load_w_small(wv, "wv_sb")
                ident_sb = wpool.tile([128, 128], F32, tag="ident")
                nc.scalar.dma_start(ident_sb[:], _ident_dram[0].ap()[:])

                kh_sb = {(b, qt): qk_pool.tile([128, N // 4], BF16,
                                               tag=f"kh{b}{qt}",
                                               name=f"kh{b}{qt}")
                         for b in range(B) for qt in range(4)}
                qh_sb = {(b, qt): qk_pool.tile([128, N // 4], BF16,
                                               tag=f"qh{b}{qt}",
                                               name=f"qh{b}{qt}")
                         for b in range(B) for qt in range(4)}
                # vh token-major with ones column per head:
                # [h0 d64 | 1 | h1 d64 | 1]
                vh_aug = [vh_pool.tile([128, HPC * (DH + 1)], BF16,
                                       tag="vh_aug", name=f"vh{g}")
                          for g in range(TOKG // 128)]

                def load_x(x_dram, b, half):
                    # alternate HWDGE (sync) / SWDGE (gpsimd) queues: the
                    # single HWDGE descriptor-gen unit (~625ns/DMA) would
                    # otherwise serialize the startup x stream
                    tiles = []
                    for cb in range(8):
                        xt = xpool.tile([128, 1024], BF16, tag="x_sb")
                        nc.sync.dma_start(
                            xt[:], x_dram[cb * 128:(cb + 1) * 128,
                                          b * N + half * 1024:
                                          b * N + (half + 1) * 1024])
                        tiles.append(xt)
                    return tiles

                # feature-major kh/qh proj for batch b into dst_sb[b]
                def proj_fmajor(x_dram, w_sb, dst_sb, b, halves=(0, 1)):
                    for half in halves:
                        xt = load_x(x_dram, b, half)
                        for q in range(2):
                            ps = psP.tile([128, 512], F32, tag="pj")
                            for cb in range(8):
                                nc.tensor.matmul(
                                    ps[:],
                                    w_sb[:, cb * 128:(cb + 1) * 128],
                                    xt[cb][:, q * 512:(q + 1) * 512],
                                    start=(cb == 0), stop=(cb == 7))
                            nc.vector.tensor_copy(
                                dst_sb[(b, half * 2 + q)][:], ps[:])

                # token-major vh proj for batch b, drains into vh_aug
                def proj_v(b):
                    for half in range(2):
                        xt = load_x(xv, b, half)
                        for s in range(2):  # psum slot: 4 token-chunks
                            ps = psP.tile([128, 512], F32, tag="pj")
                            for k in range(4):
                                t = s * 4 + k
                                for cb in range(8):
                                    nc.tensor.matmul(
                                        ps[:, k * 128:(k + 1) * 128],
                                        xt[cb][:, t * 128:(t + 1) * 128],
                                        wv_sb[:, cb * 128:(cb + 1) * 128],
                                        start=(cb == 0), stop=(cb == 7))
                            for k in range(4):
                                g = b * 16 + half * 8 + s * 4 + k
                                src = (ps[:, k * 128:(k + 1) * 128]
                                       .rearrange("p (h f) -> p h f", h=2))
                                dst = (vh_aug[g][:]
                                       .rearrange("p (h f) -> p h f", f=DH + 1)
                                       [:, :, 0:DH])
                                nc.vector.tensor_copy(dst, src)
                                ones = (vh_aug[g][:]
                                        .rearrange("p (h f) -> p h f",
                                                   f=DH + 1)[:, :, DH:DH + 1])
                                nc.vector.memset(ones, 1.0)

                def emit_bias(ic, quarters=range(4), tiles=None):
                    # 4 jb row-blocks concatenated along free per tile:
                    # one SWDGE DMA covers 4 jbs (512 descriptor rows)
                    if tiles is None:
                        tiles = [None] * 4
                    for jq in quarters:
                        ebt = bias_pool.tile([128, 4096], BF16, tag="eb")
                        src = (expb[jq * 512:(jq + 1) * 512,
                                    ic * 1024:(ic + 1) * 1024]
                               .rearrange("(s p) c -> p s c", p=128))
                        dst = ebt[:].rearrange("p (s c) -> p s c", s=4)
                        nc.gpsimd.dma_start(dst, src)
                        tiles[jq] = ebt
                    return tiles

                # quarter-granular proj (startup path): 8 [128,512] x tiles
                def proj_q(x_dram, w_sb, dst_tile, b, qt):
                    xt = []
                    for cb in range(8):
                        t = xpool.tile([128, 512], BF16, tag="x_q", bufs=8)
                        nc.sync.dma_start(
                            t[:], x_dram[cb * 128:(cb + 1) * 128,
                                         b * N + qt * 512:
                                         b * N + (qt + 1) * 512])
                        xt.append(t)
                    ps = psP.tile([128, 512], F32, tag="pj")
                    for cb in range(8):
                        nc.tensor.matmul(ps[:],
                                         w_sb[:, cb * 128:(cb + 1) * 128],
                                         xt[cb][:], start=(cb == 0),
                                         stop=(cb == 7))
                    nc.vector.tensor_copy(dst_tile[:], ps[:])

                def emit_S(ic, b, jb, eb_tiles):
                    ps = psS.tile([128, 1024], F32, tag="s")
                    kt = kh_sb[(b, jb // 4)]
                    qt = qh_sb[(b, ic)]
                    for h in range(HPC):
                        nc.tensor.matmul(
                            ps[:, h * 512:(h + 1) * 512],
                            kt[h * DH:(h + 1) * DH,
                               (jb % 4) * 128:(jb % 4 + 1) * 128],
                            qt[h * DH:(h + 1) * DH, :],
                            start=True, stop=True)
                    es = pexp_pool.tile([128, 1024], BF16, tag="es")
                    nc.scalar.activation(es[:], ps[:], _EXP)
                    nc.vector.tensor_mul(
                        es[:], es[:],
                        eb_tiles[jb // 4][:, (jb % 4) * 1024:
                                          (jb % 4 + 1) * 1024])
                    return es

                # AV in out-[i, d] form: av_ps[s] packs i-slices {2s, 2s+1}:
                # col layout [isl_a h0 65 | isl_a h1 65 | isl_b h0 | isl_b h1]
                def alloc_AV():
                    return [psA.tile([128, 512], F32, tag="av",
                                     name=f"av{s}") for s in range(2)]

                # one accumulation group = one (i-slice, head) region; the
                # 16 j-chunk matmuls of a group run CONSECUTIVELY: groups
                # interleaved within a PSUM bank corrupt each other on HW
                def emit_AV_group(st, gi):
                    ic, b, p_tiles, av_ps = st
                    s, k, h = gi // 4, (gi // 2) % 2, gi % 2
                    isl = 2 * s + k
                    for jb in range(16):
                        nc.tensor.matmul(
                            av_ps[s][:, k * 130 + h * 65:
                                     k * 130 + h * 65 + 65],
                            p_tiles[jb][:, h * 512 + isl * 128:
                                        h * 512 + isl * 128 + 128],
                            vh_aug[b * 16 + jb][:, h * 65:h * 65 + 65],
                            start=(jb == 0), stop=(jb == 15))

                def finish_AV(st):
                    ic, b, p_tiles, av_ps = st
                    nrm = [nrm_pool.tile([128, 128], F32, tag="nrm",
                                         name=f"nrm{i}") for i in range(4)]
                    for s in range(2):
                        rcp = nrm_pool.tile([128, 4], F32, tag="rcp")
                        den = (av_ps[s][:, 0:260]
                               .rearrange("p (k o) -> p k o", o=65)
                               [:, :, DH:DH + 1])
                        nc.vector.reciprocal(
                            rcp[:].rearrange("p (k o) -> p k o", o=1), den)
                        for k in range(2):
                            for h in range(HPC):
                                nc.vector.tensor_scalar_mul(
                                    nrm[2 * s + k][:, h * DH:(h + 1) * DH],
                                    av_ps[s][:, k * 130 + h * 65:
                                             k * 130 + h * 65 + DH],
                                    rcp[:, 2 * k + h:2 * k + h + 1])
                    psT = psP.tile([128, 512], F32, tag="pj")
                    for isl in range(4):
                        nc.tensor.transpose(
                            psT[:, isl * 128:(isl + 1) * 128],
                            nrm[isl][:], ident_sb[:])
                    aot = aot_pool.tile([128, 512], BF16, tag="aot")
                    nc.vector.tensor_copy(aot[:], psT[:])
                    t = 2 * b + ic // 2
                    dst = ao_in0 if ic % 2 == 0 else ao_in1
                    nc.scalar.dma_start(dst[t * 128:(t + 1) * 128, :], aot[:])

                # ---------------- stage O ----------------
                def load_wo():
                    # xpool slots: x tiles are dead by the time wo loads
                    tiles = []
                    for eb in range(8):
                        wt = xpool.tile([128, DIM], BF16, tag="x_sb",
                                        name=f"wo{eb}")
                        nc.gpsimd.dma_start(
                            wt[:], wo[eb * 128:(eb + 1) * 128, :])
                        tiles.append(wt)
                    return tiles

                def load_aon(ao_out_x, tg):
                    # single DMA: [8*128 e, 512 i] -> [128, (8 eb, 512)]
                    at = xpool.tile([128, 8 * 512], BF16, tag=tg, name=tg,
                                    bufs=1)
                    src = ao_out_x[:].rearrange("(e p) i -> p e i", p=128)
                    dst = at[:].rearrange("p (e i) -> p e i", e=8)
                    nc.gpsimd.dma_start(dst, src)
                    return at

                def stage_O_group(X, aon, wo_sb, ts, oc):
                    ps = psP.tile([128, 512], F32, tag="pj")
                    for eb in range(8):
                        nc.tensor.matmul(
                            ps[:],
                            aon[:, eb * 512 + ts * 128:
                                eb * 512 + (ts + 1) * 128],
                            wo_sb[eb][:, oc * 512:(oc + 1) * 512],
                            start=(eb == 0), stop=(eb == 7))
                    ob = od_pool.tile([128, 512], F32, tag="od")
                    nc.vector.tensor_copy(ob[:], ps[:])
                    nc.scalar.dma_start(
                        out[X * 512 + ts * 128:X * 512 + (ts + 1) * 128,
                            oc * 512:(oc + 1) * 512], ob[:])

                # ---------------- schedule ----------------
                # combo order: ic pairs {0,2} then {1,3}, b-interleaved:
                # bias for two ics SBUF-resident at a time, ao_in0 (ics
                # {0,2}) complete after combo 8 for the early half-AllToAll.
                seq = [(ic, b) for pair in ((0, 2), (1, 3))
                       for b in range(B) for ic in pair]

                # minimal work before the first exp: kh(0) + first qh chunk
                # prologue: S(0,0) jb 0-7 only needs qh(b0,q0)+kh(b0,h0); it
                # must sit BEFORE the kh-h1 proj in PE queue order or it
                # stalls on the serialized x-DMA stream. Only bias quarter 0
                # is loaded ahead of the stream.
                eb0 = emit_bias(0, quarters=(0,))
                proj_q(xq, wq_sb, qh_sb[(0, 0)], 0, 0)
                proj_q(xk, wk_sb, kh_sb[(0, 0)], 0, 0)
                proj_q(xk, wk_sb, kh_sb[(0, 1)], 0, 1)
                p0 = [emit_S(0, 0, jb, eb0) for jb in range(4)]
                eb_by_ic = {0: emit_bias(0, quarters=(1, 2, 3), tiles=eb0)}
                p0 += [emit_S(0, 0, jb, eb0) for jb in range(4, 8)]
                proj_fmajor(xk, wk_sb, kh_sb, 0, halves=(1,))
                p0 += [emit_S(0, 0, jb, eb0) for jb in range(8, 16)]
                prev = (0, 0, p0, None)

                proj_fmajor(xq, wq_sb, qh_sb, 0, halves=(1,))
                proj_q(xq, wq_sb, qh_sb[(0, 1)], 0, 1)
                proj_v(0)

                # interleaved proj work: chunk i emitted by end of window
                # deadline[i]; batch b first used at window 2b (S) / 2b+1
                # (AV).
                proj_chunks = [
                    (1, lambda: proj_fmajor(xk, wk_sb, kh_sb, 1)),
                    (1, lambda: proj_fmajor(xq, wq_sb, qh_sb, 1)),
                    (2, lambda: proj_v(1)),
                    (3, lambda: proj_fmajor(xk, wk_sb, kh_sb, 2)),
                    (3, lambda: proj_fmajor(xq, wq_sb, qh_sb, 2)),
                    (4, lambda: proj_v(2)),
                    (5, lambda: proj_fmajor(xk, wk_sb, kh_sb, 3)),
                    (5, lambda: proj_fmajor(xq, wq_sb, qh_sb, 3)),
                    (6, lambda: proj_v(3)),
                ]

                wo_sb = None
                aon0 = None
                o0_groups = [(ts, oc) for ts in range(4) for oc in range(2)]
                for n in range(1, len(seq) + 1):
                    prev = (prev[0], prev[1], prev[2], alloc_AV())
                    fin_ic_b = (prev[0], prev[1])
                    if n < len(seq):
                        ic, b = seq[n]
                        if ic not in eb_by_ic:
                            eb_by_ic[ic] = emit_bias(ic)
                        p_cur = []
                        for jb in range(16):
                            p_cur.append(emit_S(ic, b, jb, eb_by_ic[ic]))
                            if jb % 2 == 1:
                                emit_AV_group(prev, jb // 2)
                        finish_AV(prev)
                        prev = (ic, b, p_cur, None)
                        while proj_chunks and proj_chunks[0][0] <= n:
                            proj_chunks.pop(0)[1]()
                        if False and n >= 13 and aon0 is not None and "O" in stages:
                            # ride phase-2 PE slack with X0 out-proj groups
                            for _ in range(3):
                                if o0_groups:
                                    ts, oc = o0_groups.pop(0)
                                    stage_O_group(0, aon0, wo_sb, ts, oc)
                    else:
                        for gi in range(8):
                            emit_AV_group(prev, gi)
                        finish_AV(prev)
                        a2a(ao_in1, ao_out1)
                        if "O" in stages:
                            aon1 = load_aon(ao_out1, "aon1")
                            for ts, oc in o0_groups:
                                stage_O_group(0, aon0, wo_sb, ts, oc)
                            for ts in range(4):
                                for oc in range(2):
                                    stage_O_group(1, aon1, wo_sb, ts, oc)
                    if fin_ic_b == (2, 3):
                        # all ic {0,2} blocks written -> ship first half
                        if dbg_dump:
                            nc.sync.dma_start(dbg[:], ao_in0[:])
                        a2a(ao_in0, ao_out0)
                        wo_sb = load_wo()
                        aon0 = load_aon(ao_out0, "aon0")

    nc.compile()
    return nc


def make_in_maps(q, k, v, rel_pos_bias, Wq, Wk, Wv, Wo):
    """Host-side sharding: transposes, bf16 casts, exp(bias) packing."""
    q = np.asarray(q, np.float32).reshape(TOKG, DIM)
    k = np.asarray(k, np.float32).reshape(TOKG, DIM)
    v = np.asarray(v, np.float32).reshape(TOKG, DIM)
    bias = np.asarray(rel_pos_bias, np.float32)
    xq_t = np.ascontiguousarray(q.T).astype(NP_BF16)
    xk_t = np.ascontiguousarray(k.T).astype(NP_BF16)
    xv_t = np.ascontiguousarray(v.T).astype(NP_BF16)
    wq_t = np.ascontiguousarray((np.asarray(Wq) * SCALE).T).astype(NP_BF16)
    wk_t = np.ascontiguousarray(np.asarray(Wk).T).astype(NP_BF16)
    wv_t = np.ascontiguousarray(np.asarray(Wv).T).astype(NP_BF16)
    wo_t = np.ascontiguousarray(np.asarray(Wo).T).astype(NP_BF16)
    in_maps = []
    for c in range(NCORES):
        hs = slice(HPC * DH * c, HPC * DH * (c + 1))
        # [h, i, j] -> exp -> packed [j, (ic, h, iw)] with i = ic*512 + iw
        eb = np.exp(bias[0, HPC * c:HPC * (c + 1)])          # [h, i, j]
        eb = eb.transpose(2, 1, 0)                            # [j, i, h]
        eb = eb.reshape(N, 4, 512, 2).transpose(0, 1, 3, 2)   # [j, ic, h, iw]
        in_maps.append({
            "xq": xq_t, "xk": xk_t, "xv": xv_t,
            "wq": np.ascontiguousarray(wq_t[:, hs]),
            "wk": np.ascontiguousarray(wk_t[:, hs]),
            "wv": np.ascontiguousarray(wv_t[:, hs]),
            "wo": wo_t,
            "expb": np.ascontiguousarray(eb.reshape(N, 2 * N)).astype(NP_BF16),
        })
    return in_maps


_NC_CACHE = None


def kernel(q, k, v, rel_pos_bias, Wq, Wk, Wv, Wo):
    global _NC_CACHE
    if _NC_CACHE is None:
        _NC_CACHE = build_nc()
    nc = _NC_CACHE
    in_maps = make_in_maps(q, k, v, rel_pos_bias, Wq, Wk, Wv, Wo)
    res = bass_utils.run_bass_kernel_spmd(nc, in_maps,
                                          core_ids=list(range(NCORES)))
    out = np.empty((B, N, DIM), np.float32)
    for c in range(NCORES):
        b, half = c // 2, c % 2
        out[b, half * TOK:(half + 1) * TOK, :] = res.results[c]["out"]
    return out
